# revision 41
# baseline (speedup 1.0000x reference)
"""Trainium2 Bass kernel for fused MHA block + mean-pool (nn_MemoryFusion).

Computes, for X [4, 2048, 2048] bf16 and per-tensor weights/biases:
    Q/K/V = X @ W* + b*          (per-head split, 16 heads of dk=128)
    A     = softmax(Q K^T / sqrt(dk))
    out   = mean_s(concat_heads(A @ V) @ Wo + bo)   -> [4, 2048]

Sharding: tensor-parallel over heads; each of the 8 cores owns 2 heads.
The host sums the 8 partial [4, 2048] results and adds bo.

v3 key algebra: only the mean over queries is needed, so
    mean_q(A @ V) = w @ V = w @ (X @ Wv) + bv = ((w @ X) @ Wv) + bv
with w = colsum_q(A)/S.  This removes BOTH the V projection and the
dense attn@V matmuls.  The colsum of A is computed on the tensor
engine as dn-stationary 1-column-weight matmuls over the exp'd scores
(same PE cost as attn@V), the V projection disappears entirely, and
w@X is a stream of 1/2-column matmuls over X tiles.

Per (b, h) attention, scores are produced in [q-part, k-free]
orientation so the softmax denominators come for free from the scalar
engine's fused activation accumulator (accum_out of exp), killing the
old DVE add-tree.

Phases per core (2 heads of the 16):
  P(b): Q/K projection of batch b -> qkt [dk, {q0,q1,k0,k1}, S]
  A(b): per (h, qc in 16): scores [128q, 2048k] -> exp (+denominator
        accum) -> reciprocal -> colsum matmuls into pw (psum, packed
        4x[1,512] per bank via base-partition offsets)
  Z(b): pw -> (DMA transpose) -> w16 [k%128, kb, h]; z = w @ X via
        256 ap-2 matmuls on xn tiles; maT = z @ Wv_head + bv
  F   : out = maT^T @ Wo rows (mean folded), host adds bo and reduces.

P(b+1) groups and Z(b-1) groups are interleaved into A(b)'s PE stream
as fill work so the tensor engine never parks.
"""

import numpy as np
import ml_dtypes

import concourse.bass as bass
import concourse.mybir as mybir
import concourse.tile as tile
from concourse.bass_utils import run_bass_kernel_spmd

P = 128
B = 4
S = 2048
D = 2048
H_PER_CORE = 2
DK = 128
E = H_PER_CORE * DK          # 256: per-core qkv output slice
ND = D // P                  # 16 contraction chunks
NQC = S // P                 # 16 q-chunks per head
N_CORES = 8

BF16 = mybir.dt.bfloat16
F32 = mybir.dt.float32
FP8 = mybir.dt.float8e4
WQK_SCALE = 32.0
COLSUM_DR = False  # DR needs multi-col stationary; 1-col dn weight not encodable

# 1/sqrt(dk) as the reference computes it (rounded through bf16)
SCALE = float(np.asarray(1.0 / np.sqrt(DK), dtype=ml_dtypes.bfloat16))


class SplitDrainTileContext(tile.TileContext):
    """TileContext emitting at most one sem wait per instruction.

    The walrus build in this toolchain rejects >1 sync wait on any TPB
    instruction; upstream Tile can attach several. Split the extras onto
    same-engine NoOp carriers inserted right before the instruction (and
    onto extra Drains for the tail drain).
    """

    def _lower_ordered_insts(self, ordered):
        for bb_name, insts in ordered.items():
            new_list = []
            for inst in insts:
                si = inst.sync_info
                if si is not None and len(si.on_wait) > 1:
                    waits = list(si.on_wait)
                    for k, w in enumerate(waits[:-1]):
                        nop = mybir.InstNoOp(name=f"{inst.name}-sw{k}",
                                             ins=[], outs=[])
                        nop.engine = inst.engine
                        nop.sync_info = mybir.SyncInfo(on_wait=[w],
                                                       on_update=[])
                        new_list.append(nop)
                    inst.sync_info = mybir.SyncInfo(
                        on_wait=[waits[-1]], on_update=list(si.on_update))
                new_list.append(inst)
            ordered[bb_name] = new_list
        return super()._lower_ordered_insts(ordered)

    def _drain_and_barrier(self, tick_clock, wait_clock):
        from concourse.vector_clock import ScopedClock

        d = self.nc.sync.drain()
        wait_clock.add_sem_waits(d.ins, ScopedClock({None: tick_clock.global_clock}))
        si = d.ins.sync_info
        if si is not None and len(si.on_wait) > 1:
            waits = list(si.on_wait)
            d.ins.sync_info = mybir.SyncInfo(
                on_wait=[waits[0]], on_update=list(si.on_update)
            )
            for w in waits[1:]:
                d2 = self.nc.sync.drain()
                d2.ins.sync_info = mybir.SyncInfo(on_wait=[w], on_update=[])
        self.nc.all_engine_barrier()
        popped = self.nc._tile_sem_poison_stack.pop()
        assert popped is self._sem_poison
        self.nc.clear_and_free_semaphores(list(self.sems.allocated().values()))
        self.nc.all_engine_barrier()


def build_nc(repeat=1):
    nc = bass.Bass("TRN2", target_bir_lowering=False, debug=False,
                   num_devices=N_CORES)

    xt = nc.dram_tensor("xt", [B, D, S], FP8, kind="ExternalInput")
    xn = nc.dram_tensor("xn", [B, ND, P, ND, P], BF16, kind="ExternalInput")
    wqk = nc.dram_tensor("wqk", [P, 4 * ND, P], FP8, kind="ExternalInput")
    wv = nc.dram_tensor("wv", [P, ND, E], BF16, kind="ExternalInput")
    wo = nc.dram_tensor("wo", [P, H_PER_CORE, D], BF16, kind="ExternalInput")
    bqk = nc.dram_tensor("bqk", [P, 4], F32, kind="ExternalInput")
    bv = nc.dram_tensor("bv", [P, H_PER_CORE], F32, kind="ExternalInput")
    out = nc.dram_tensor("out", [B, D], F32, kind="ExternalOutput")

    ident = mybir.ActivationFunctionType.Identity
    expf = mybir.ActivationFunctionType.Exp
    copyf = mybir.ActivationFunctionType.Copy
    addop = mybir.AluOpType.add
    mulop = mybir.AluOpType.mult

    with SplitDrainTileContext(nc) as tc:
        with (
            tc.tile_pool(name="const", bufs=1) as cpool,
            tc.tile_pool(name="xt", bufs=ND // 2) as xt_pool,
            tc.tile_pool(name="qkt", bufs=2) as qkt_pool,
            tc.tile_pool(name="es", bufs=3) as es_pool,
            tc.tile_pool(name="xn", bufs=6) as xn_pool,
            tc.tile_pool(name="den", bufs=2) as den_pool,
            tc.tile_pool(name="wbar", bufs=2) as wbar_pool,
            tc.tile_pool(name="pqkv", bufs=1, space="PSUM") as pqkv_pool,
            tc.tile_pool(name="ps", bufs=2, space="PSUM") as ps_pool,
            tc.tile_pool(name="pw", bufs=1, space="PSUM") as pw_pool,
            tc.tile_pool(name="paux", bufs=1, space="PSUM") as paux_pool,
        ):
            # ---- constants / weights resident in SBUF ----
            wqk_s = cpool.tile([P, 4 * ND, P], FP8)   # [d%128, (eb,dchunk), e%128]
            for eb in (2, 0, 3, 1):
                nc.sync.dma_start(wqk_s[:, eb * ND:(eb + 1) * ND, :],
                                  wqk[:, eb * ND:(eb + 1) * ND, :])
            # wv/wo DMAs are deferred into _body (after batch 0's xt
            # tiles) -- they are only needed by the Z/final phases and
            # would otherwise queue ahead of the startup-critical xt
            wv_s = cpool.tile([P, ND, E], BF16)        # [d%128, dchunk, e]
            wo_s = cpool.tile([P, H_PER_CORE, D], BF16)
            bqk_s = cpool.tile([P, 4], F32)
            nc.sync.dma_start(bqk_s[:], bqk[:])
            bv_s = cpool.tile([P, H_PER_CORE], F32)
            nc.sync.dma_start(bv_s[:], bv[:])
            mln16_s = cpool.tile([P, 1], F32)
            nc.vector.memset(mln16_s[:], -float(np.log(16.0)))
            maT16 = cpool.tile([P, H_PER_CORE, B], BF16)
            outsb = cpool.tile([B, D], F32)

            import contextlib
            loop_cm = (tc.For_i(0, repeat, 1) if repeat > 1
                       else contextlib.nullcontext())
            with loop_cm:
                _body(nc, tc, locals())

    return nc


def _body(nc, tc, env):
    (cpool, xt_pool, qkt_pool, es_pool, xn_pool, den_pool, wbar_pool,
     pqkv_pool, ps_pool, pw_pool, paux_pool) = (
        env[k] for k in ("cpool", "xt_pool", "qkt_pool", "es_pool", "xn_pool",
                         "den_pool", "wbar_pool", "pqkv_pool", "ps_pool",
                         "pw_pool", "paux_pool"))
    wqk_s, wv_s, wo_s, bqk_s, bv_s, maT16, outsb, mln16_s = (
        env[k] for k in ("wqk_s", "wv_s", "wo_s", "bqk_s", "bv_s", "maT16",
                         "outsb", "mln16_s"))
    xt, xn, out = env["xt"], env["xn"], env["out"]
    ident = env["ident"]; expf = env["expf"]; copyf = env["copyf"]
    addop = env["addop"]; mulop = env["mulop"]

    NKB = 4          # k-blocks of 512 per colsum row
    KW = S // NKB    # 512

    def stage_p(b):
        """Allocate tiles + DMA for batch b's Q/K projection; return
        (qkt, groups).  Group order is k0, q0, k1, q1 (eb 2,0,3,1) so
        attention on head h can start as soon as its two eb rows land."""
        xt_tiles = []
        for dp in range(ND // 2):
            t = xt_pool.tile([P, 2, S], FP8, tag="xt")
            for u in range(2):
                nc.sync.dma_start(
                    t[:, u, :], xt[b, (2 * dp + u) * P:(2 * dp + u + 1) * P, :])
            xt_tiles.append(t)

        qkt = qkt_pool.tile([P, 4, S], BF16, tag="qkt")

        def qkt_group(eb, sb):
            # emitted as micro-thunks so each DoubleRow matmul's 256-col
            # weight load hides under an attention matmul's 512-col
            # stream (the DR groups are LDWEIGHTS-bound on hardware)
            state = {}

            def mm(dp):
                def emit():
                    if dp == 0:
                        psq = pqkv_pool.tile([P, 512], F32, tag="pqkv")
                        state["ps"] = psq
                    nc.tensor.matmul(
                        state["ps"][:],
                        wqk_s[:, eb * ND + 2 * dp:eb * ND + 2 * dp + 2, :],
                        xt_tiles[dp][:, :, sb * 512:(sb + 1) * 512],
                        start=(dp == 0), stop=(dp == ND // 2 - 1),
                        perf_mode=mybir.MatmulPerfMode.DoubleRow,
                    )
                return emit

            def evac():
                # on DVE: ACT is the critical engine (exp stream)
                with nc.allow_low_precision("qkt evac to bf16"):
                    nc.vector.tensor_scalar(
                        qkt[:, eb, sb * 512:(sb + 1) * 512], state["ps"][:],
                        1.0 / WQK_SCALE, bqk_s[:, eb:eb + 1],
                        mulop, addop,
                    )

            def emit():
                microq.extend(mm(dp) for dp in range(ND // 2))
                microq.append(evac)
            return emit

        groups = []
        for eb in (2, 0, 3, 1):
            for sb in range(4):
                groups.append(qkt_group(eb, sb))
        return qkt, groups

    # deferred PE work (colsum of the previous (h, qc) step)
    pending = []

    def flush_pending():
        while pending:
            pending.pop(0)()

    from collections import deque
    microq = deque()

    def micro(n=1):
        for _ in range(n):
            if microq:
                microq.popleft()()

    # fill queue entries: (pe_cost_us, min_step, thunk).  stage_a pops
    # entries whose min_step has been reached, paced by a PE-cost
    # credit so heavy thunks spread evenly over the 32 attention steps.
    fillq = deque()

    W_GROUP = 3.4   # 16 x 512-col matmuls
    W_ZGRP = 0.45   # 16 x 2-col matmuls (ldweights-bound on hw)
    W_WV = 0.25

    def stage_a(b, qkt, harvest, extra_units=0.0):
        """Attention for batch b: per (h, qc) scores -> exp(+den accum)
        -> recip -> (deferred) colsum into pw.  `harvest(h, pw)` is
        called right after head h's last colsum is flushed."""
        steps = H_PER_CORE * NQC
        total_units = sum(w for w, _, _ in fillq) + extra_units
        per_step = total_units / steps
        credit = 0.0
        step = 0
        pw_tiles = []
        for h in range(H_PER_CORE):
            # dn8[:, t, qp]: fp8 of 256*16/den for qc = 2*qp + t (16B
            # stride between pair halves, as DoubleRow weights need)
            dn8 = den_pool.tile([P, 2, NQC // 2], FP8, tag=f"dn{h % 2}")
            dnf = den_pool.tile([P, NQC], F32, tag=f"dnf{h % 2}")
            # pw row 32j holds colsum block j, k-linear free layout
            # (c*128 + p with c = kb within block, p = k%128)
            pw = pw_pool.tile([P, NKB, P], F32, tag=f"pw{h % 2}")
            pw_tiles.append(pw)
            for qc in range(NQC):
                qs = slice(qc * P, (qc + 1) * P)
                qp, tq = qc // 2, qc % 2
                den2 = den_pool.tile([P, 2], F32, tag="den2")
                if tq == 0:
                    es = es_pool.tile([P, 2, S], FP8, tag="es")
                for t in range(2):
                    ps = ps_pool.tile([P, 1024], F32, tag="ps")
                    for u in range(2):
                        kb2 = 2 * t + u
                        nc.tensor.matmul(
                            ps[:, u * 512:(u + 1) * 512],
                            qkt[:, h, qs],
                            qkt[:, 2 + h, kb2 * 512:(kb2 + 1) * 512],
                            start=True, stop=True,
                        )
                        micro()
                    # es' = exp(s/sqrt(dk))/16 in fp8 (max ~15 < e4m3
                    # 240); accum gives den/16 and (16/den)(es/16)
                    # cancels exactly.
                    nc.scalar.activation(
                        es[:, tq, t * 1024:(t + 1) * 1024], ps[:], expf,
                        scale=SCALE, bias=mln16_s[:, 0:1],
                        accum_out=den2[:, t:t + 1],
                    )
                dsum = den_pool.tile([P, 1], F32, tag="dsum")
                nc.vector.tensor_tensor(dsum[:], den2[:, 0:1], den2[:, 1:2],
                                        addop)
                nc.vector.reciprocal(dnf[:, qc:qc + 1], dsum[:])
                with nc.allow_low_precision("softmax 1/den as fp8"):
                    # 256/den' in fp8 (~O(1)); 1/256 unfolds at pw evac
                    nc.vector.tensor_scalar_mul(dn8[:, tq, qp:qp + 1],
                                                dnf[:, qc:qc + 1], 256.0)

                def colsum(qp=qp, es=es, dn8=dn8, pw=pw):
                    for j in range(NKB):
                        ks = slice(j * KW, (j + 1) * KW)
                        if COLSUM_DR:
                            nc.tensor.matmul(
                                pw[32 * j:32 * j + 1, :, :],
                                dn8[:, :, qp:qp + 1],
                                es[:, :, ks],
                                start=(qp == 0), stop=(qp == NQC // 2 - 1),
                                tile_position=(0, 32 * j),
                                perf_mode=mybir.MatmulPerfMode.DoubleRow,
                            )
                        else:
                            for t2 in range(2):
                                nc.tensor.matmul(
                                    pw[32 * j:32 * j + 1, :, :],
                                    dn8[:, t2, qp:qp + 1],
                                    es[:, t2, ks],
                                    start=(qp == 0 and t2 == 0),
                                    stop=(qp == NQC // 2 - 1 and t2 == 1),
                                    tile_position=(0, 32 * j),
                                )
                                micro()
                if tq == 1:
                    pending.append(colsum)
                    if len(pending) > 1:
                        pending.pop(0)()
                credit += per_step
                while (fillq and fillq[0][1] <= step
                       and credit + 1e-9 >= fillq[0][0]):
                    w, _, t = fillq.popleft()
                    credit -= w
                    t()
                step += 1
            if pending:
                pending.pop(0)()
            harvest(h, pw)
        while fillq:
            fillq.popleft()[2]()
            micro(len(microq))
        micro(len(microq))
        return pw_tiles

    xn_books = {}

    def make_harvest(b):
        """Per-batch w-tiles plus the harvest(h, pw) closure: psum pw ->
        sbuf (DVE), DMA de-interleave into wT [k%128, kb, h] (pw free
        layout is f' = p*NKB + c so the source is contiguous), bf16
        cast.  All non-PE engines, emitted as soon as head h's colsum
        accumulation completes."""
        wT = wbar_pool.tile([P, ND, H_PER_CORE], F32, tag="wT")
        w16 = wbar_pool.tile([P, ND, H_PER_CORE], BF16, tag="w16")
        zsb = wbar_pool.tile([P, ND, H_PER_CORE], BF16, tag="zsb")
        pwsb = wbar_pool.tile([1, H_PER_CORE, NKB, NKB, P], F32,
                              tag="pwsb", bufs=1)
        tiles = {"wT": wT, "w16": w16, "zsb": zsb, "pwsb": pwsb}

        def harvest(h, pw):
            for j in range(NKB):
                # [1, NKB, 128] psum row -> sbuf (DVE; Pool can't read
                # PSUM); unfolds the 256x dn8 scale
                nc.vector.tensor_scalar_mul(
                    pwsb[0:1, h, j, :, :], pw[32 * j:32 * j + 1, :, :],
                    1.0 / 256.0)
            for j in range(NKB):
                for c in range(NKB):
                    # contiguous 128-element column -> partition spread
                    nc.sync.dma_start(wT[:, NKB * j + c, h],
                                      pwsb[0:1, h, j, c, :])
            nc.vector.tensor_scalar_mul(w16[:, :, h], wT[:, :, h], 1.0)

        xn_tiles = {}
        xn_books[b] = xn_tiles

        def xn_dma(dc):
            def emit():
                t = xn_pool.tile([P, ND, P], BF16, tag="xn")
                nc.sync.dma_start(t[:], xn[b, dc])
                xn_tiles[dc] = t
            return emit
        tiles["xn_dma"] = xn_dma
        return tiles, harvest

    def stage_z(b, tiles):
        """Weighted thunks for z = w @ X and maT = z @ Wv + bv."""
        thunks = []
        w16 = tiles["w16"]
        zsb = tiles["zsb"]

        pz = paux_pool.tile([P, ND, H_PER_CORE], F32, tag="paux")
        xn_tiles = xn_books.pop(b)

        def z_group(dc):
            def emit():
                t = xn_tiles.pop(dc)
                for kb in range(ND):
                    nc.tensor.matmul(
                        pz[:, dc, :],
                        t[:, kb, :],
                        w16[:, kb, :],
                        start=(kb == 0), stop=(kb == ND - 1),
                    )
            return emit

        def z_evac():
            nc.vector.tensor_scalar_mul(zsb[:], pz[:], 1.0)

        def wv_apply(h):
            def emit():
                pv = paux_pool.tile([P, 1], F32, tag="paux")
                for dc in range(ND):
                    nc.tensor.matmul(
                        pv[:],
                        wv_s[:, dc, h * DK:(h + 1) * DK],
                        zsb[:, dc, h:h + 1],
                        start=(dc == 0), stop=(dc == ND - 1),
                    )
                with nc.allow_low_precision("maT16 evac to bf16"):
                    nc.vector.tensor_scalar(
                        maT16[:, h, b:b + 1], pv[:], 1.0 / S,
                        bv_s[:, h:h + 1], mulop, addop)
            return emit

        for dc in range(ND):
            thunks.append((W_ZGRP, z_group(dc)))
        thunks.append((0.0, z_evac))
        thunks.append((W_WV, wv_apply(0)))
        thunks.append((W_WV, wv_apply(1)))
        return thunks

    # ---------------- main schedule ----------------
    # One batch ahead: P(b+1) groups and Z(b-1) thunks fill batch b's
    # attention.  For b >= 1, stage_p(b+1) is called directly at the top
    # of iteration b (all P(b) reads of the xt slots are already
    # emitted, so the xt DMAs can start immediately).  For b = 0 the
    # P(1) emission must wait until P(0)'s queued groups are in the
    # stream, so it rides a zero-cost spawn thunk.
    qkts = {}
    qkts[0], g0 = stage_p(0)
    nc.sync.dma_start(wv_s[:], env["wv"][:])
    nc.sync.dma_start(wo_s[:], env["wo"][:])
    # serial prefix: k0 row + first q0 chunk; h=0 attention can then
    # start while the rest stream in as fills
    for g in g0[:5]:
        g()
        micro(len(microq))
    for i, g in enumerate(g0[5:]):
        fillq.append((W_GROUP, max(0, i - 2), g))

    def spawn1():
        q, gs = stage_p(1)
        qkts[1] = q
        for i, g in enumerate(gs):
            fillq.append((W_GROUP, 12 + round(1.3 * i), g))
    fillq.append((0.0, 0, spawn1))

    zprev = []
    for b in range(B):
        if b >= 1 and b + 1 < B:
            qkts[b + 1], gs = stage_p(b + 1)
        else:
            gs = []
        tiles, harvest = make_harvest(b)
        entries = [(w, 2 * (i // 2), t) for i, (w, t) in enumerate(zprev)]
        entries += [(W_GROUP, 4 + round(1.6 * i), g) for i, g in enumerate(gs)]
        entries += [(0.0, 16 + dc // 2, tiles["xn_dma"](dc))
                    for dc in range(ND)]
        entries.sort(key=lambda e: e[1])
        fillq.extend(entries)
        extra = W_GROUP * 16 if b == 0 else 0.0
        pw_tiles = stage_a(b, qkts.pop(b), harvest, extra_units=extra)
        zprev = stage_z(b, tiles)
    for _, t in zprev:
        t()

    # ---- final: outsb = maT^T @ Wo rows (mean already folded) ----
    for nb in range(4):
        ns = slice(nb * 512, (nb + 1) * 512)
        pf = paux_pool.tile([B, 512], F32, tag="paux")
        for h in range(H_PER_CORE):
            nc.tensor.matmul(pf[:], maT16[:, h, :], wo_s[:, h, ns],
                             start=(h == 0), stop=(h == H_PER_CORE - 1))
        nc.vector.tensor_scalar_mul(outsb[:, ns], pf[:], 1.0)
    nc.sync.dma_start(out[:], outsb[:])

    return nc


def _shard_inputs(X, Wq, bq, Wk, bk, Wv, bv, Wo, bo):
    """Build the 8 per-core input maps (numpy, bf16)."""
    bf = ml_dtypes.bfloat16
    X = np.asarray(X, dtype=bf)
    Wq, Wk, Wv, Wo = (np.asarray(w, dtype=bf) for w in (Wq, Wk, Wv, Wo))
    bq, bk, bv, bo = (np.asarray(v, dtype=bf) for v in (bq, bk, bv, bo))

    f8 = ml_dtypes.float8_e4m3
    xt = np.ascontiguousarray(X.transpose(0, 2, 1)).astype(f8)  # [B, D, S]
    # xn[b, dc, p, kb, j] = X[b, kb*128+p, dc*128+j]
    xn = np.ascontiguousarray(
        X.reshape(B, ND, P, ND, P).transpose(0, 3, 2, 1, 4))

    in_maps = []
    for c in range(N_CORES):
        es = slice(c * E, (c + 1) * E)
        # [d, e] slices -> [128, (eb, dchunk), 128] with eb-major free dim
        wq_c = Wq[:, es].reshape(ND, P, 2, DK)   # [dchunk, d%128, eb, e%128]
        wk_c = Wk[:, es].reshape(ND, P, 2, DK)
        wqk_c = np.concatenate([wq_c, wk_c], axis=2)      # eb: q0,q1,k0,k1
        wqk_c = np.ascontiguousarray(wqk_c.transpose(1, 2, 0, 3)).reshape(
            P, 4 * ND, P)                                  # [(d%128),(eb,dc),e]
        wqk_c = (wqk_c.astype(np.float32) * WQK_SCALE).astype(f8)
        wv_c = np.ascontiguousarray(
            Wv[:, es].reshape(ND, P, E).transpose(1, 0, 2))  # [128, dchunk, e]
        wo_c = np.ascontiguousarray(
            Wo[es, :].reshape(H_PER_CORE, P, D).transpose(1, 0, 2))
        bqk_c = np.ascontiguousarray(
            np.concatenate([bq[es], bk[es]]).astype(np.float32).reshape(4, P).T)  # [128, 4]
        bv_c = np.ascontiguousarray(
            bv[es].reshape(H_PER_CORE, P).T.astype(np.float32))
        in_maps.append({
            "xt": xt, "xn": xn, "wqk": wqk_c, "wv": wv_c, "wo": wo_c,
            "bqk": bqk_c, "bv": bv_c,
        })
    return in_maps, np.asarray(bo, dtype=np.float32)


_CACHED_NC = None


def kernel(X, Wq, bq, Wk, bk, Wv, bv, Wo, bo):
    global _CACHED_NC
    in_maps, bo_f32 = _shard_inputs(X, Wq, bq, Wk, bk, Wv, bv, Wo, bo)
    if _CACHED_NC is None:
        _CACHED_NC = build_nc()
    res = run_bass_kernel_spmd(_CACHED_NC, in_maps, list(range(N_CORES)))
    total = np.zeros((B, D), dtype=np.float32)
    for c in range(N_CORES):
        total += res.results[c]["out"]
    total += bo_f32
    return total.astype(ml_dtypes.bfloat16)
